# revision 1
# baseline (speedup 1.0000x reference)
"""Contrastive loss (InfoNCE-style logsumexp of cosine-similarity matrix) on
8 Trainium2 NeuronCores.

loss = -mean_i logsumexp_j( cos(z1_i, z2_j) / 0.05 ),  z1,z2: [8192, 512] f32

Strategy: shard z1 row-wise (1024 rows/core), replicate z2. Each core, fully
pipelined at supergroup (8-row-block / 1024-column) granularity:
  1. streams z2 in 1 MiB batched DMAs; row sum-of-squares split across ACT
     (fused Square+accum) and GpSimd-mult + DVE-reduce; batched Sqrt + DVE
     reciprocal; row-scale on GpSimd,
  2. z1 is fed RAW (its 20/||z1_i|| scale is applied later as the
     per-partition Exp scale); PE-transposes both to d-major layout through
     2-bank PSUM tiles; batched PSUM->SBUF copies convert to float32r
     (rounded fp32, 11-bit mantissa -> 4x faster PE datapath),
  3. sim block-row: f32r matmuls (K=512 via 4 accumulating chunks) into
     [128, 1024] 2-bank PSUM tiles, emitted right after the z2 columns they
     need are ready, so the PE alternates transposes and matmuls with no
     phase barriers,
  4. one ACT Exp per tile, in place, scale=20/||z1_i||, fused row-sum
     (accum_out); logsumexp without max-subtraction (|sim| <= 20 ->
     exp <= 5e8, safe in fp32),
  5. reduce + Ln -> per-row lse [128, 8] -> DRAM.
Host gathers the 8 lse tiles and returns -mean.
"""
import sys

sys.path.insert(0, "/opt/trn_rl_repo")
import numpy as np
import concourse.bacc as bacc
import concourse.mybir as mybir
from concourse import tile, masks
from concourse.bass_utils import run_bass_kernel_spmd

F32 = mybir.dt.float32
F32R = mybir.dt.float32r
AF = mybir.ActivationFunctionType
ALU = mybir.AluOpType

N, D, C = 8192, 512, 8
NS = N // C            # 1024 z1 rows per core
IB = NS // 128         # 8 i-blocks per core
NB2 = N // 128         # 64 z2 row-blocks
JH = 8                 # j-supergroups of 1024 columns (2-bank PSUM tiles)
INV_TEMP = 20.0        # 1 / 0.05


def _build():
    nc = bacc.Bacc("TRN2", target_bir_lowering=False, debug=False, num_devices=C)
    z1_d = nc.dram_tensor("z1s", [NS, D], F32, kind="ExternalInput").ap()
    z2_d = nc.dram_tensor("z2", [N, D], F32, kind="ExternalInput").ap()
    lse_d = nc.dram_tensor("lse", [128, IB], F32, kind="ExternalOutput").ap()

    with tile.TileContext(nc) as tc:
        with (
            tc.tile_pool(name="const", bufs=1) as cpool,
            tc.tile_pool(name="stage", bufs=4) as stg,
            tc.tile_pool(name="hat", bufs=4) as hat,
            tc.tile_pool(name="sqs", bufs=2) as sqs,
            tc.tile_pool(name="pbig", bufs=4, space="PSUM") as pbig,
        ):
            ident = cpool.tile([128, 128], F32)
            masks.make_identity(nc, ident[:])

            z1T = cpool.tile([128, 4 * NS], F32R, name="z1T")    # [d, (k, i)]
            z2T = cpool.tile([128, 4 * N], F32R, name="z2T")     # [d, (k, j)]
            z1Tk = z1T[:].rearrange("p (k i) -> p k i", k=4)
            z2Tk = z2T[:].rearrange("p (k j) -> p k j", k=4)
            z1Tb = z1T[:].rearrange("p (k nb i) -> p nb k i", k=4, i=128)
            z2Tb = z2T[:].rearrange("p (k nb i) -> p nb k i", k=4, i=128)

            n1sq = cpool.tile([128, IB], F32, name="n1sq")
            n1s = cpool.tile([128, IB], F32, name="n1s")
            rn1 = cpool.tile([128, IB], F32, name="rn1")
            n2sq = cpool.tile([128, NB2], F32, name="n2sq")
            n2s = cpool.tile([128, NB2], F32, name="n2s")
            rn2 = cpool.tile([128, NB2], F32, name="rn2")
            esums = cpool.tile([128, IB * JH], F32, name="esums")
            stot = cpool.tile([128, IB], F32, name="stot")
            lse_s = cpool.tile([128, IB], F32, name="lse_s")

            psv = "p (nb k i) -> p nb k i"

            def sumsq(st, n, nsq_col, b):
                blk = st[:, n * D:(n + 1) * D]
                sq = sqs.tile([128, D], F32, tag="sq", name="sq_scr")
                if b % 2 == 0:
                    nc.scalar.activation(sq[:], blk, AF.Square, accum_out=nsq_col)
                else:
                    nc.gpsimd.tensor_mul(sq[:], blk, blk)
                    nc.vector.reduce_sum(nsq_col, sq[:], axis=mybir.AxisListType.X)

            def transpose2(src_aps, name):
                # 8 PE transposes (2 row-blocks x 4 d-chunks) -> one 2-bank tile
                ps = pbig.tile([128, 1024], F32, tag="big", name=name)
                for n in range(2):
                    for k in range(4):
                        nc.tensor.transpose(
                            ps[:, (n * 4 + k) * 128:(n * 4 + k + 1) * 128],
                            src_aps[n][:, k * 128:(k + 1) * 128], ident[:])
                return ps

            z1r = z1_d.rearrange("(g n p) d -> g p n d", n=4, p=128)
            z2r = z2_d.rearrange("(g n p) d -> g p n d", n=4, p=128)
            z2st = {}

            def z1_group(g):
                # raw transposes straight off the staged tile (no normalize)
                st = stg.tile([128, 4 * D], F32, tag="stage", name=f"st1_{g}")
                nc.sync.dma_start(out=st[:].rearrange("p (n d) -> p n d", n=4),
                                  in_=z1r[g])
                for h in range(2):
                    b0 = 4 * g + 2 * h
                    ps = transpose2([st[:, (2 * h) * D:(2 * h + 1) * D],
                                     st[:, (2 * h + 1) * D:(2 * h + 2) * D]],
                                    f"ps1_{g}_{h}")
                    nc.scalar.copy(z1Tb[:, b0:b0 + 2],
                                   ps[:].rearrange(psv, nb=2, k=4))
                for n in range(4):
                    sumsq(st, n, n1sq[:, 4 * g + n:4 * g + n + 1], 4 * g + n)

            def z2_load(g):
                st = stg.tile([128, 4 * D], F32, tag="stage", name=f"st2_{g}")
                nc.sync.dma_start(out=st[:].rearrange("p (n d) -> p n d", n=4),
                                  in_=z2r[g])
                z2st[g] = st
                for n in range(4):
                    sumsq(st, n, n2sq[:, 4 * g + n:4 * g + n + 1], 4 * g + n)

            def z2_finish(gs):
                s = slice(4 * gs[0], 4 * gs[-1] + 4)
                nc.scalar.activation(n2s[:, s], n2sq[:, s], AF.Sqrt)
                nc.vector.reciprocal(rn2[:, s], n2s[:, s])
                for gg in gs:
                    st = z2st.pop(gg)
                    zhs = []
                    for n in range(4):
                        b = 4 * gg + n
                        zh = hat.tile([128, D], F32, tag="hat", name="zh")
                        nc.gpsimd.tensor_scalar(
                            zh[:], st[:, n * D:(n + 1) * D],
                            rn2[:, b:b + 1], 1.0, op0=ALU.mult, op1=ALU.mult)
                        zhs.append(zh)
                    for h in range(2):
                        b0 = 4 * gg + 2 * h
                        ps = transpose2(zhs[2 * h:2 * h + 2], f"ps2_{gg}_{h}")
                        nc.vector.tensor_copy(z2Tb[:, b0:b0 + 2],
                                              ps[:].rearrange(psv, nb=2, k=4))

            def main_tile(ib, jh):
                # [128, 1024] sim tile: 2 j-groups of 512, K=512 via 4 chunks
                ps = pbig.tile([128, 1024], F32, tag="big", name=f"mm{ib}_{jh}")
                for k in range(4):
                    for jq in range(2):
                        jb = jh * 2 + jq
                        nc.tensor.matmul(
                            ps[:, jq * 512:(jq + 1) * 512],
                            lhsT=z1Tk[:, k, ib * 128:(ib + 1) * 128],
                            rhs=z2Tk[:, k, jb * 512:(jb + 1) * 512],
                            start=(k == 0), stop=(k == 3),
                            skip_group_check=True)
                nc.scalar.activation(
                    ps[:], ps[:], AF.Exp, scale=rn1[:, ib:ib + 1],
                    accum_out=esums[:, ib * JH + jh:ib * JH + jh + 1])

            # ---------- emission: z2-first startup, then supergroup stream
            z2_load(0)
            z2_load(1)
            z2_finish([0, 1])
            z1_group(0)
            z1_group(1)
            # rn1 = 20 / ||z1_i||: sqrt(nsq/400) then reciprocal
            nc.scalar.activation(n1s[:], n1sq[:], AF.Sqrt, scale=1.0 / 400.0)
            nc.vector.reciprocal(rn1[:], n1s[:])
            for ib in range(IB):
                main_tile(ib, 0)
            for g in range(2, 2 * JH):
                z2_load(g)
                if g % 2 == 1:
                    z2_finish([g - 1, g])
                    jh = g // 2
                    for ib in range(IB):
                        main_tile(ib, jh)

            # ---------- logsumexp tail
            nc.vector.reduce_sum(stot[:], esums[:].rearrange("p (a b) -> p a b", b=JH),
                                 axis=mybir.AxisListType.X)
            nc.scalar.activation(lse_s[:], stot[:], AF.Ln)
            nc.sync.dma_start(out=lse_d[:], in_=lse_s[:])

    nc.compile()
    return nc


_nc = None


def _get_nc():
    global _nc
    if _nc is None:
        _nc = _build()
    return _nc


def kernel(z1: np.ndarray, z2: np.ndarray, _trace: bool = False, **_):
    nc = _get_nc()
    z1 = np.ascontiguousarray(z1, dtype=np.float32)
    z2 = np.ascontiguousarray(z2, dtype=np.float32)
    in_maps = [
        {"z1s": z1[c * NS:(c + 1) * NS], "z2": z2} for c in range(C)
    ]
    res = run_bass_kernel_spmd(nc, in_maps, list(range(C)), trace=_trace)
    total = 0.0
    for c in range(C):
        total += res.results[c]["lse"].astype(np.float64).sum()
    out = np.float32(-(total / N))
    if _trace:
        return out, res
    return out



# revision 12
# speedup vs baseline: 1.2780x; 1.2780x over previous
"""Contrastive loss (InfoNCE-style logsumexp of cosine-similarity matrix) on
8 Trainium2 NeuronCores.

loss = -mean_i logsumexp_j( cos(z1_i, z2_j) / 0.05 ),  z1,z2: [8192, 512] f32

Strategy: shard z1 row-wise (1024 rows/core), replicate z2. Each core runs a
3-wave-deep pipeline over 8 supergroups (sg) of 1024 z2 rows:
  1. DMA sg (2 MiB) -> DVE bn_stats/bn_aggr row stats (one pass; sumsq =
     512*(var+mean^2), the ln 512 folded into the exp-table bias) -> ACT
     Ln+Exp gives rn2 = S2/||z2_j|| (rsqrt inside the exp table: no Sqrt
     table thrash) -> GpSimd tensor_scalar normalize+cast to fp8e4,
  2. PE transposes the fp8 rows to d-major (1 cyc/row); fp8 transpose mode
     writes PSUM at element step 2, so the PSUM->SBUF copies move uint16
     granules (DVE 2x mode) into step-2 fp8 z1T/z2T layouts,
  3. sim block-row: fp8 DoubleRow matmuls (K=256/instr, 0.5 cyc/row) read
     the stride-2 fp8 operands directly into [128, 1024] 2-bank PSUM tiles,
  4. one ACT Exp per tile, in place, scale = rn1_i = 20/(S1*S2*||z1_i||),
     fused row-sum (accum_out); logsumexp without max-subtraction
     (|logits| <= 20 -> exp <= 5e8, safe in fp32),
  5. reduce + Ln -> per-row lse [128, 8] -> DRAM.
z1 is quantized raw (x S1) to fp8; its norm is folded into the Exp scale.
Host gathers the 8 lse tiles and returns -mean.
"""
import sys

sys.path.insert(0, "/opt/trn_rl_repo")
import math
import numpy as np
import concourse.bacc as bacc
import concourse.mybir as mybir
from concourse import tile, masks
from concourse.bass_utils import run_bass_kernel_spmd

F32 = mybir.dt.float32
FP8 = mybir.dt.float8e4
U16 = mybir.dt.uint16
AF = mybir.ActivationFunctionType
ALU = mybir.AluOpType
DR = mybir.MatmulPerfMode.DoubleRow

N, D, C = 8192, 512, 8
NS = N // C            # 1024 z1 rows per core
IB = NS // 128         # 8 i-blocks per core
SG = 8                 # z2 supergroups of 1024 rows
BPG = 8                # 128-row blocks per supergroup
S1 = 16.0              # z1 fp8 scale
S2 = 64.0              # z2-hat fp8 scale
# rn = exp(-.5 ln(var+mean^2) + B); sumsq = D*(var+mean^2)
B1 = math.log(20.0 / (S1 * S2)) - 0.5 * math.log(D)
B2 = math.log(S2) - 0.5 * math.log(D)


def _build():
    nc = bacc.Bacc("TRN2", target_bir_lowering=False, debug=False, num_devices=C)
    z1_d = nc.dram_tensor("z1s", [NS, D], F32, kind="ExternalInput").ap()
    z2_d = nc.dram_tensor("z2", [N, D], F32, kind="ExternalInput").ap()
    lse_d = nc.dram_tensor("lse", [128, IB], F32, kind="ExternalOutput").ap()

    with tile.TileContext(nc) as tc:
        with (
            tc.tile_pool(name="const", bufs=1) as cpool,
            tc.tile_pool(name="stage", bufs=3) as stg,
            tc.tile_pool(name="c8", bufs=3) as c8p,
            tc.tile_pool(name="sqs", bufs=4) as sqs,
            tc.tile_pool(name="pmm", bufs=3, space="PSUM") as pmm,
            tc.tile_pool(name="ptp", bufs=2, space="PSUM") as ptp,
        ):
            ident = cpool.tile([128, 128], FP8)
            masks.make_identity(nc, ident[:])
            b1t = cpool.tile([128, 1], F32, name="b1c")
            b2t = cpool.tile([128, 1], F32, name="b2c")
            nc.gpsimd.memset(b1t[:], B1)
            nc.gpsimd.memset(b2t[:], B2)

            # step-2 fp8 transposed operands, stored as uint16 granules
            z1T = cpool.tile([128, 4, NS], U16, name="z1T")    # [d, k, i]
            z2T = cpool.tile([128, 4, N], U16, name="z2T")     # [d, k, j]

            def f8view(t, kp, sl):
                v = t[:, 2 * kp:2 * kp + 2, sl].bitcast(FP8)
                return v.rearrange("p k (x two) -> p k x two", two=2)[:, :, :, 0]

            n1sq = cpool.tile([128, IB], F32, name="n1sq")
            n1l = cpool.tile([128, IB], F32, name="n1l")
            rn1 = cpool.tile([128, IB], F32, name="rn1")
            n1mv = cpool.tile([128, IB, 2], F32, name="n1mv")
            n2sq = cpool.tile([128, SG * BPG], F32, name="n2sq")
            n2l = cpool.tile([128, SG * BPG], F32, name="n2l")
            rn2 = cpool.tile([128, SG * BPG], F32, name="rn2")
            n2mv = cpool.tile([128, SG * BPG, 2], F32, name="n2mv")
            z1c = cpool.tile([128, IB, D], FP8, name="z1c")
            esums = cpool.tile([128, IB * SG], F32, name="esums")
            stot = cpool.tile([128, IB], F32, name="stot")
            lse_s = cpool.tile([128, IB], F32, name="lse_s")

            z1r = z1_d.rearrange("(n p) d -> p n d", n=IB, p=128)
            z2r = z2_d.rearrange("(g n p) d -> g p n d", n=BPG, p=128)

            def rowstats(st_view, b, mv_slice):
                stats = sqs.tile([128, 6], F32, tag="sq", name="bn_scr")
                nc.vector.bn_stats(out=stats[:], in_=st_view[:, b])
                nc.vector.bn_aggr(out=mv_slice, in_=stats[:])

            def finish_norms(mv, cols, nsq, nl, rn, bias):
                # nsq = mean^2 + var (batched); rn = exp(-.5 ln nsq + bias)
                mvv = mv[:, cols, :]
                nc.vector.tensor_mul(nsq[:, cols], mvv[:, :, 0], mvv[:, :, 0])
                nc.vector.tensor_add(nsq[:, cols], nsq[:, cols], mvv[:, :, 1])
                nc.scalar.activation(nl[:, cols], nsq[:, cols], AF.Ln)
                nc.scalar.activation(rn[:, cols], nl[:, cols], AF.Exp,
                                     scale=-0.5, bias=bias)

            z2c8 = {}

            def prep_load(g):
                # DMA + row stats + rn2 + normalize/cast for supergroup g
                st = stg.tile([128, BPG, D], F32, tag="stage", name=f"st2_{g}")
                nc.sync.dma_start(out=st[:], in_=z2r[g])
                stv = st[:]
                for b in range(BPG):
                    rowstats(stv, b, n2mv[:, g * BPG + b, :])
                cols = slice(g * BPG, (g + 1) * BPG)
                finish_norms(n2mv[:], cols, n2sq, n2l, rn2, b2t[:])
                c8 = c8p.tile([128, BPG, D], FP8, tag="c8", name=f"c8_{g}")
                z2c8[g] = c8
                for b in range(BPG):
                    nc.gpsimd.tensor_scalar(
                        c8[:, b], stv[:, b],
                        rn2[:, g * BPG + b:g * BPG + b + 1], 1.0,
                        op0=ALU.mult, op1=ALU.mult)

            def tp_unit(src, kh, jh, dst, name):
                # transpose 4 row-blocks x 2 d-chunks -> one 1-bank PSUM
                # stage (fp8 transpose writes at element step 2), then one
                # uint16 DVE copy (2x mode) into the step-2 z1T/z2T layout
                ps = ptp.tile([128, 2, 512, 2], FP8, tag="tp", name=name)
                for kk in range(2):
                    k = 2 * kh + kk
                    for bl in range(4):
                        b = 4 * jh + bl
                        nc.tensor.transpose(
                            ps[:, kk, bl * 128:(bl + 1) * 128, 0],
                            src[:, b, k * 128:(k + 1) * 128], ident[:])
                src16 = ps[:].rearrange("p k j t -> p k (j t)").bitcast(U16)
                nc.vector.tensor_copy(dst, src16)

            def prep_tp(g):
                c8 = z2c8.pop(g)
                for kh in range(2):
                    for jh in range(2):
                        sl = slice(g * 1024 + jh * 512, g * 1024 + jh * 512 + 512)
                        tp_unit(c8[:], kh, jh,
                                z2T[:, 2 * kh:2 * kh + 2, sl],
                                f"tp2_{g}_{kh}{jh}")

            def main_tile(g, ib):
                ps = pmm.tile([128, 1024], F32, tag="mm", name=f"mm{g}_{ib}")
                for kp in range(2):
                    for jq in range(2):
                        nc.tensor.matmul(
                            ps[:, jq * 512:(jq + 1) * 512],
                            lhsT=f8view(z1T, kp,
                                        slice(ib * 128, (ib + 1) * 128)),
                            rhs=f8view(z2T, kp,
                                       slice(g * 1024 + jq * 512,
                                             g * 1024 + jq * 512 + 512)),
                            start=(kp == 0), stop=(kp == 1),
                            perf_mode=DR, skip_group_check=True)
                col = ib * SG + g
                nc.scalar.activation(
                    ps[:], ps[:], AF.Exp, scale=rn1[:, ib:ib + 1],
                    accum_out=esums[:, col:col + 1])

            # ---------- prologue
            prep_load(0)
            prep_load(1)

            # z1: DMA, row stats, rn1, cast to fp8, transpose
            st1 = stg.tile([128, IB, D], F32, tag="z1st", name="st1")
            nc.sync.dma_start(out=st1[:], in_=z1r)
            st1v = st1[:]
            for b in range(IB):
                rowstats(st1v, b, n1mv[:, b, :])
            finish_norms(n1mv[:], slice(0, IB), n1sq, n1l, rn1, b1t[:])
            z1cv = z1c[:]
            for b in range(IB):
                nc.gpsimd.tensor_scalar(z1cv[:, b], st1v[:, b], S1, 1.0,
                                        op0=ALU.mult, op1=ALU.mult)
            for kh in range(2):
                for jh in range(2):
                    tp_unit(z1cv, kh, jh,
                            z1T[:, 2 * kh:2 * kh + 2,
                                jh * 512:jh * 512 + 512], f"tp1_{kh}{jh}")

            prep_load(2)
            prep_tp(0)
            prep_tp(1)

            # ---------- main loop
            for g in range(SG):
                if g + 3 < SG:
                    prep_load(g + 3)
                if g + 2 < SG:
                    # interleave next-next supergroup's transposes between
                    # matmul tiles so the PE never stalls head-of-line
                    for ib in range(3):
                        main_tile(g, ib)
                    prep_tp(g + 2)
                    for ib in range(3, IB):
                        main_tile(g, ib)
                else:
                    for ib in range(IB):
                        main_tile(g, ib)

            # ---------- logsumexp tail
            nc.vector.reduce_sum(
                stot[:], esums[:].rearrange("p (a b) -> p a b", b=SG),
                axis=mybir.AxisListType.X)
            nc.scalar.activation(lse_s[:], stot[:], AF.Ln)
            nc.sync.dma_start(out=lse_d[:], in_=lse_s[:])

    nc.compile()
    return nc


_nc = None


def _get_nc():
    global _nc
    if _nc is None:
        _nc = _build()
    return _nc


def kernel(z1: np.ndarray, z2: np.ndarray, _trace: bool = False, **_):
    nc = _get_nc()
    z1 = np.ascontiguousarray(z1, dtype=np.float32)
    z2 = np.ascontiguousarray(z2, dtype=np.float32)
    in_maps = [
        {"z1s": z1[c * NS:(c + 1) * NS], "z2": z2} for c in range(C)
    ]
    res = run_bass_kernel_spmd(nc, in_maps, list(range(C)), trace=_trace)
    total = 0.0
    for c in range(C):
        total += res.results[c]["lse"].astype(np.float64).sum()
    out = np.float32(-(total / N))
    if _trace:
        return out, res
    return out
